# revision 23
# baseline (speedup 1.0000x reference)
"""AttentionBlock kernel for Trainium2, data-parallel over batch on 8 NeuronCores.

Per-core computation (one batch element, x_b: [256, 4096] = [C, H*W]):
  GroupNorm(8 groups) folded into the QKV projection:
    xn = x*scale_c + shift_c   (per-channel affine from group stats)
    qkv = W_qkv xn + b  ==  (W_qkv * scale_c) x + (W_qkv shift + b)
  Everything heavy runs as fp8e4m3 DoubleRow matmuls: each instruction
  contracts 2 k-tiles (K=256) at ~1.7x the f32r rate.  During the x DMA,
  the fp8 cast of x doubles as the GN sum (ACT accum_out) while the DVE
  computes the sum of squares (tensor_tensor_reduce), so GN stats are
  ready as soon as x lands.  q,k land in fp8 [128, 2, N]; v goes out
  transposed as vT8 [128, MT, C] with the v-bias added via a precomputed
  broadcast row and scaled by 1/8 so the attention numerator stays in
  e4m3 range.  QKV matmul/eviction units are interleaved so attention can
  start while late eviction chunks still drain.
  Attention core (per 512-token query chunk nb, per key-tile pair j):
    S'[m,n] = sum_c k8[c,m] q8[c,n]   (1 DoubleRow matmul per m-tile)
    P' = exp(S'/16 - 2.5) -> fp8      (the -2.5 bias keeps P' in e4m3
                                       range and cancels in P'/d)
    out[c,n] += vT8 pair @ P' pair    (DoubleRow over m-tile pairs)
    d[n]    += ones8 pair @ P' pair   (ones8 = 0.125 on all 128 rows: the
                                       denominator lands pre-broadcast
                                       across partitions and pre-scaled)
  S/exp emission runs 2 pairs ahead of out/d so the ACT latency hides, and
  the prefetch continues across nb boundaries (the epilogue of chunk nb is
  emitted between the first S pairs of chunk nb+1).
  Normalization is deferred past the proj matmul (it commutes):
    y = (proj8(att8) * (1/d)) + proj_b + x
"""

import sys

sys.path.insert(0, "/opt/trn_rl_repo")

import ml_dtypes
import numpy as np

import concourse.bass as bass  # noqa: F401
import concourse.mybir as mybir
import concourse.tile as tile
from concourse import bacc
from concourse.bass_utils import run_bass_kernel_spmd

F32 = mybir.dt.float32
F32R = mybir.dt.float32r
F8 = mybir.dt.float8e4
DR = mybir.MatmulPerfMode.DoubleRow
AF = mybir.ActivationFunctionType
ALU = mybir.AluOpType

C = 256
N = 4096
GROUPS = 8
EPS = 1e-5
CT = 2          # channel tiles of 128
MT = 32         # m (key/token) tiles of 128
NB = 8          # n (query/token) chunks of 512
NCHUNK = 512
SCALE = 1.0 / 16.0  # 1/sqrt(C)
EXPB = -2.5         # exp bias: P' = exp(S/16 - 2.5), keeps fp8e4 in range
VSCALE = 0.125      # v scaled by 1/8 into fp8 so att=P'@v stays under 240;
                    # ones8 = VSCALE so the same factor lands in d and cancels
GSIZE = C // GROUPS
GN_COUNT = float(GSIZE * N)
XCH = 2         # x DMA/stat chunks per c-tile
XCW = N // XCH  # 2048
NPAIR = MT // 2
PREF = 2        # S/exp pairs emitted ahead of out/d accumulation


def _build():
    nc = bacc.Bacc("TRN2", target_bir_lowering=False)

    x_d = nc.declare_dram_parameter("x", [C, N], F32, isOutput=False)
    wqkvT_d = nc.declare_dram_parameter("wqkvT", [C, 3 * C], F32R, isOutput=False)
    wpT8_d = nc.declare_dram_parameter("wpT8", [128, 2 * C], F8, isOutput=False)
    bqk_d = nc.declare_dram_parameter("bqk", [128, 4], F32, isOutput=False)
    bvrow_d = nc.declare_dram_parameter("bvrow", [1, C], F32, isOutput=False)
    bp_d = nc.declare_dram_parameter("bp", [128, 2], F32, isOutput=False)
    gamma_d = nc.declare_dram_parameter("gamma", [128, 2], F32, isOutput=False)
    beta_d = nc.declare_dram_parameter("beta", [128, 2], F32, isOutput=False)
    sel_d = nc.declare_dram_parameter("sel", [128, 2 * GROUPS], F32, isOutput=False)
    selb_d = nc.declare_dram_parameter("selb", [GROUPS, C], F32, isOutput=False)
    ones_d = nc.declare_dram_parameter("ones", [128, 128], F32R, isOutput=False)
    out_d = nc.declare_dram_parameter("out", [C, N], F32, isOutput=True)
    dbg_d = nc.declare_dram_parameter("dbg", [1, 2], F32, isOutput=True)

    with tile.TileContext(nc) as tc:
        with (
            tc.tile_pool(name="const", bufs=1) as cp,
            tc.tile_pool(name="work", bufs=1) as wp,
            nc.allow_low_precision("f32r accumulators hold exact f32 bits"),
        ):
            ones = cp.tile([128, 128], F32R, name="ones", tag="ones")
            nc.sync.dma_start(ones[:], ones_d[:])
            ones8 = cp.tile([128, 2, 128], F8, name="ones8", tag="ones8")
            nc.any.memset(ones8[:], VSCALE)
            bias_exp = cp.tile([128, 1], F32, name="bias_exp", tag="bias_exp")
            nc.any.memset(bias_exp[:], EXPB)
            # dummy activations preload the Sqrt/Exp ACT tables while the x
            # DMA is in flight; DMA'd to a debug output so they survive DCE
            dumm = cp.tile([1, 1], F32, name="dumm", tag="dumm")
            nc.any.memset(dumm[:], 1.0)
            dumo = cp.tile([1, 2], F32, name="dumo", tag="dumo")
            nc.scalar.activation(dumo[:, 0:1], dumm[:], AF.Sqrt)
            nc.scalar.activation(dumo[:, 1:2], dumm[:], AF.Exp)
            nc.sync.dma_start(dbg_d[:], dumo[:])
            # ---- x loads, chunk-major so casts/stats start early ----
            xt = [cp.tile([128, N], F32, name=f"x{t}", tag=f"x{t}") for t in range(CT)]
            for ch in range(XCH):
                for t in range(CT):
                    nc.sync.dma_start(xt[t][:, ch * XCW:(ch + 1) * XCW],
                                      x_d[t * 128:(t + 1) * 128, ch * XCW:(ch + 1) * XCW])
            # ---- fp8 cast doubles as GN sum (ACT accum); DVE does sum-sq ----
            x8 = cp.tile([128, CT, N], F8, name="x8", tag="x8")
            stats = [cp.tile([128, 2 * XCH], F32, name=f"stats{t}", tag=f"stats{t}")
                     for t in range(CT)]
            for ch in range(XCH):
                for t in range(CT):
                    xv = xt[t][:, ch * XCW:(ch + 1) * XCW]
                    nc.vector.tensor_reduce(stats[t][:, ch:ch + 1], xv,
                                            mybir.AxisListType.X, ALU.add)
                    scratch = wp.tile([128, XCW], F32, tag="scratch", name="scratch")
                    nc.scalar.activation(scratch[:], xv, AF.Square,
                                         accum_out=stats[t][:, XCH + ch:XCH + ch + 1])

            # ---- remaining loads ----
            wT = []
            for t in range(CT):
                wtile = cp.tile([128, 3 * C], F32R, name=f"wT{t}", tag=f"wT{t}")
                nc.sync.dma_start(wtile[:], wqkvT_d[t * 128:(t + 1) * 128, :])
                wT.append(wtile)
            wp8t = cp.tile([128, 2, C], F8, name="wpT8", tag="wpT8")
            nc.sync.dma_start(wp8t[:], wpT8_d[:])
            bqk = cp.tile([128, 4], F32, name="bqk", tag="bqk")
            nc.sync.dma_start(bqk[:], bqk_d[:])
            bvrow = cp.tile([1, C], F32, name="bvrow", tag="bvrow")
            nc.sync.dma_start(bvrow[:], bvrow_d[:])
            bp = cp.tile([128, 2], F32, name="bp", tag="bp")
            nc.sync.dma_start(bp[:], bp_d[:])
            gamma = cp.tile([128, 2], F32, name="gamma", tag="gamma")
            nc.sync.dma_start(gamma[:], gamma_d[:])
            beta = cp.tile([128, 2], F32, name="beta", tag="beta")
            nc.sync.dma_start(beta[:], beta_d[:])
            sel = cp.tile([128, 2 * GROUPS], F32, name="sel", tag="sel")
            nc.sync.dma_start(sel[:], sel_d[:])
            selb = cp.tile([GROUPS, C], F32, name="selb", tag="selb")
            nc.sync.dma_start(selb[:], selb_d[:])

            # ---- setup-phase PSUM pool (closed before the QKV pool) ----
            with tc.tile_pool(name="ps0", bufs=2, space="PSUM") as ps0:
                # dummy matmuls keep the PE HAM-warm while DMA + GN stats run
                for wi in range(56):
                    wps = ps0.tile([128, 128], F32, tag="warm", name="wps")
                    nc.tensor.matmul(wps[:], ones[:], ones[:], start=True, stop=True)
                g_ps = ps0.tile([GROUPS, 2 * XCH], F32, tag="small", name="g_ps")
                nc.tensor.matmul(g_ps[:], sel[:, 0:GROUPS], stats[0][:], start=True, stop=False)
                nc.tensor.matmul(g_ps[:], sel[:, GROUPS:2 * GROUPS], stats[1][:], start=False, stop=True)
                # per-group mean / rstd on partitions 0..7
                g_mr = cp.tile([GROUPS, 2], F32, name="g_mr", tag="g_mr")
                gtmp = cp.tile([GROUPS, 5], F32, name="gtmp", tag="gtmp")
                g_sb = cp.tile([GROUPS, 2 * XCH], F32, name="g_sb", tag="g_sb")
                nc.vector.tensor_copy(g_sb[:], g_ps[:])
                nc.vector.tensor_add(gtmp[:, 3:4], g_sb[:, 0:1], g_sb[:, 1:2])
                nc.vector.tensor_add(gtmp[:, 4:5], g_sb[:, 2:3], g_sb[:, 3:4])
                nc.vector.tensor_scalar_mul(g_mr[:, 0:1], gtmp[:, 3:4], 1.0 / GN_COUNT)
                nc.vector.tensor_scalar_mul(gtmp[:, 0:1], gtmp[:, 4:5], 1.0 / GN_COUNT)
                nc.vector.tensor_mul(gtmp[:, 1:2], g_mr[:, 0:1], g_mr[:, 0:1])
                nc.vector.tensor_sub(gtmp[:, 2:3], gtmp[:, 0:1], gtmp[:, 1:2])
                gvar = cp.tile([GROUPS, 1], F32, name="gvar", tag="gvar")
                nc.vector.tensor_scalar_add(gvar[:], gtmp[:, 2:3], EPS)
                gstd = cp.tile([GROUPS, 1], F32, name="gstd", tag="gstd")
                nc.scalar.activation(gstd[:], gvar[:], AF.Sqrt)
                nc.vector.reciprocal(g_mr[:, 1:2], gstd[:])

                # broadcast group mean/rstd to per-channel scale/shift
                scale_t = []
                shift_t = []
                for t in range(CT):
                    mr_ps = ps0.tile([128, 2], F32, tag="small", name="mr_ps")
                    nc.tensor.matmul(mr_ps[:], selb[:, t * 128:(t + 1) * 128], g_mr[:],
                                     start=True, stop=True)
                    mr = cp.tile([128, 2], F32, name=f"mr{t}", tag=f"mr{t}")
                    nc.vector.tensor_copy(mr[:], mr_ps[:])
                    sc = cp.tile([128, 1], F32, name=f"scale{t}", tag=f"scale{t}")
                    nc.vector.tensor_mul(sc[:], mr[:, 1:2], gamma[:, t:t + 1])
                    tmp = cp.tile([128, 1], F32, name=f"mscale{t}", tag=f"mscale{t}")
                    nc.vector.tensor_mul(tmp[:], mr[:, 0:1], sc[:])
                    # shift duplicated to 2 cols: f32r matmuls need even N
                    sh = cp.tile([128, 2], F32R, name=f"shift{t}", tag=f"shift{t}")
                    nc.vector.tensor_sub(sh[:, 0:1], beta[:, t:t + 1], tmp[:])
                    nc.vector.tensor_sub(sh[:, 1:2], beta[:, t:t + 1], tmp[:])
                    scale_t.append(sc)
                    shift_t.append(sh)

                # adjusted fp8 qkv weights: wadj8[c, t, o] = wT[c, o] * scale_c
                wadj8 = cp.tile([128, 2, 3 * C], F8, name="wadj8", tag="wadj8")
                for t in range(CT):
                    nc.vector.tensor_scalar_mul(wadj8[:, t:t + 1, :],
                                                wT[t][:].bitcast(F32), scale_t[t][:])
                # x8 casts emitted only now: in-order queues would otherwise
                # run them before the GN-chain ops they don't depend on
                for ch in range(XCH):
                    for t in range(CT):
                        xv = xt[t][:, ch * XCW:(ch + 1) * XCW]
                        cdst = x8[:, t:t + 1, ch * XCW:(ch + 1) * XCW]
                        if ch == 0:
                            nc.scalar.copy(cdst, xv)
                        else:
                            nc.vector.tensor_copy(cdst, xv)
                # q/k bias: btot[o] = qkv_b[o] + sum_c wT[c,o]*shift_c  (o in 0..512)
                bias_ps = ps0.tile([128, 4, 2], F32, tag="small", name="bias_ps")
                for ot in range(4):
                    for t in range(CT):
                        nc.tensor.matmul(bias_ps[:, ot:ot + 1, :],
                                         wT[t][:, ot * 128:(ot + 1) * 128],
                                         shift_t[t][:],
                                         start=(t == 0), stop=(t == CT - 1))
                btot = cp.tile([128, 4], F32, name="btot", tag="btot")
                nc.vector.tensor_add(btot[:], bias_ps[:, :, 0:1], bqk[:])
                # v bias row doubled: bvtot2[1, 2C] then broadcast [128, 2C] * VSCALE
                bv_ps = ps0.tile([2, C], F32, tag="small", name="bv_ps")
                for t in range(CT):
                    nc.tensor.matmul(bv_ps[:], shift_t[t][:], wT[t][:, 2 * C:3 * C],
                                     start=(t == 0), stop=(t == CT - 1))
                bvtot2 = cp.tile([1, 2 * C], F32R, name="bvtot2", tag="bvtot2")
                for h in range(2):
                    nc.vector.tensor_add(bvtot2[:, h * C:(h + 1) * C], bv_ps[0:1, :], bvrow[:])
                bv2_ps = ps0.tile([128, 2 * C], F32, tag="bv2", name="bv2_ps")
                nc.tensor.matmul(bv2_ps[:], ones[0:1, :], bvtot2[:], start=True, stop=True)
                bv2s = cp.tile([128, 2 * C], F32, name="bv2s", tag="bv2s")
                nc.vector.tensor_scalar_mul(bv2s[:], bv2_ps[:], VSCALE)

            with tc.tile_pool(name="ps1", bufs=1, space="PSUM") as ps1:
                # ---- QKV projections (fp8 DR): q,k -> fp8 [128, 2, N] ----
                q8 = cp.tile([128, CT, N], F8, name="q8", tag="q8")
                k8 = cp.tile([128, CT, N], F8, name="k8", tag="k8")
                vT8 = cp.tile([128, MT, C], F8, name="vT8", tag="vT8")
                dests = [(q8, 0), (q8, 1), (k8, 0), (k8, 1)]

                def emit_qk(ot, mcp, eng):
                    qk_ps = ps1.tile([128, 2 * NCHUNK], F32, tag="qk", bufs=3, name="qk_ps")
                    for half in range(2):
                        mc = 2 * mcp + half
                        nc.tensor.matmul(qk_ps[:, half * NCHUNK:(half + 1) * NCHUNK],
                                         wadj8[:, :, ot * 128:(ot + 1) * 128],
                                         x8[:, :, mc * NCHUNK:(mc + 1) * NCHUNK],
                                         start=True, stop=True, perf_mode=DR)
                    dtile, dt_ = dests[ot]
                    dst = dtile[:, dt_:dt_ + 1, 2 * mcp * NCHUNK:(2 * mcp + 2) * NCHUNK]
                    if eng == "act":
                        nc.scalar.activation(dst, qk_ps[:], AF.Identity,
                                             bias=btot[:, ot:ot + 1])
                    else:
                        nc.vector.tensor_scalar_add(dst, qk_ps[:], btot[:, ot:ot + 1])

                def emit_vt(mtp):
                    vt_ps = ps1.tile([128, 2 * C], F32, tag="vt", bufs=2, name="vt_ps")
                    for half in range(2):
                        mt = 2 * mtp + half
                        nc.tensor.matmul(vt_ps[:, half * C:(half + 1) * C],
                                         x8[:, :, mt * 128:(mt + 1) * 128],
                                         wadj8[:, :, 2 * C:3 * C],
                                         start=True, stop=True, perf_mode=DR)
                    nc.vector.scalar_tensor_tensor(
                        vT8[:, 2 * mtp:2 * mtp + 2, :], in0=vt_ps[:], scalar=VSCALE,
                        in1=bv2s[:], op0=ALU.mult, op1=ALU.add)

                # qk units first (q chunk0 and k earliest, on the faster DVE
                # evictor), then vT units; attention consumes them in order
                emit_qk(0, 0, "dve"); emit_qk(1, 0, "dve")
                emit_qk(2, 0, "dve"); emit_qk(3, 0, "dve")
                emit_qk(2, 1, "act"); emit_qk(3, 1, "act")
                emit_qk(2, 2, "act"); emit_qk(3, 2, "act")
                emit_qk(2, 3, "act"); emit_qk(3, 3, "act")
                for mtp in range(MT // 2):
                    emit_vt(mtp)
                # late q chunks: first consumed by nb2, ~37us into attention,
                # so they drain on DVE behind the vT evictions, keeping ACT
                # free for the first exps
                emit_qk(0, 1, "dve"); emit_qk(1, 1, "dve")
                emit_qk(0, 2, "dve"); emit_qk(1, 2, "dve")
                emit_qk(0, 3, "dve"); emit_qk(1, 3, "dve")

            with tc.tile_pool(name="ps", bufs=1, space="PSUM") as ps:
                # ---- attention: fp8 DoubleRow core, software-pipelined ----
                total = NB * NPAIR
                p8_of = {}

                def emit_s_exp(idx):
                    nb, j = divmod(idx, NPAIR)
                    nsl = slice(nb * NCHUNK, (nb + 1) * NCHUNK)
                    p8 = wp.tile([128, 2, NCHUNK], F8, tag="p", bufs=4, name="p8")
                    s_ps = ps.tile([128, 2 * NCHUNK], F32, tag="s", bufs=2, name="s_ps")
                    for i in range(2):
                        mb = 2 * j + i
                        nc.tensor.matmul(s_ps[:, i * NCHUNK:(i + 1) * NCHUNK],
                                         k8[:, :, mb * 128:(mb + 1) * 128],
                                         q8[:, :, nsl],
                                         start=True, stop=True, perf_mode=DR)
                    # one ACT exp per m-tile pair: halves ACT instruction count
                    nc.scalar.activation(p8[:], s_ps[:], AF.Exp,
                                         bias=bias_exp[:, 0:1], scale=SCALE)
                    p8_of[idx] = p8

                emitted = 0

                def prefetch(upto):
                    nonlocal emitted
                    while emitted <= min(upto, total - 1):
                        emit_s_exp(emitted)
                        emitted += 1

                for nb in range(NB):
                    nsl = slice(nb * NCHUNK, (nb + 1) * NCHUNK)
                    out_ps = [ps.tile([128, NCHUNK], F32, tag="out", bufs=3, name=f"outp{_t}")
                              for _t in range(CT)]
                    d_ps = ps.tile([128, NCHUNK], F32, tag="d", bufs=1, name="d_ps")
                    base = nb * NPAIR
                    for j in range(NPAIR):
                        idx = base + j
                        prefetch(idx + PREF)
                        p_cur = p8_of.pop(idx)
                        first, last = (j == 0), (j == NPAIR - 1)
                        for t in range(CT):
                            nc.tensor.matmul(out_ps[t][:],
                                             vT8[:, 2 * j:2 * j + 2, t * 128:(t + 1) * 128],
                                             p_cur[:], start=first, stop=last,
                                             perf_mode=DR)
                        nc.tensor.matmul(d_ps[:], ones8[:], p_cur[:],
                                         start=first, stop=last, perf_mode=DR)
                    # ---- epilogue: evict, normalize via deferred 1/d, proj ----
                    last_nb = (nb == NB - 1)
                    att8 = wp.tile([128, CT, NCHUNK], F8, tag="att", bufs=2, name="att8")
                    for t in range(CT):
                        nc.vector.tensor_copy(att8[:, t:t + 1, :], out_ps[t][:])
                    dsb = wp.tile([128, NCHUNK], F32, tag="dsb", bufs=2, name="dsb")
                    if last_nb:  # ACT is idle at the tail; shortens the exit chain
                        nc.scalar.copy(dsb[:], d_ps[:])
                    else:
                        nc.vector.tensor_copy(dsb[:], d_ps[:])
                    zsb = []
                    for ot in range(CT):
                        z_ps = ps.tile([128, NCHUNK], F32, tag="out", bufs=3, name="z_ps")
                        nc.tensor.matmul(z_ps[:],
                                         wp8t[:, :, ot * 128:(ot + 1) * 128],
                                         att8[:], start=True, stop=True, perf_mode=DR)
                        zt = wp.tile([128, NCHUNK], F32, tag="z", bufs=3, name="zsb")
                        if last_nb:
                            nc.scalar.copy(zt[:], z_ps[:])
                        else:
                            nc.vector.tensor_copy(zt[:], z_ps[:])
                        zsb.append(zt)
                    rdb = wp.tile([128, NCHUNK], F32, tag="rdb", bufs=2, name="rdb")
                    if nb < NB - 1:
                        nc.vector.reciprocal(rdb[:], dsb[:])
                        for ot in range(CT):
                            y = wp.tile([128, NCHUNK], F32, tag="y", bufs=6, name="y")
                            nc.vector.tensor_mul(y[:], zsb[ot][:], rdb[:])
                            nc.vector.scalar_tensor_tensor(
                                y[:], in0=y[:], scalar=bp[:, ot:ot + 1],
                                in1=xt[ot][:, nsl], op0=ALU.add, op1=ALU.add)
                            nc.sync.dma_start(out_d[ot * 128:(ot + 1) * 128, nsl], y[:])
                    else:
                        # last chunk: halve the normalize/residual chain so the
                        # kernel tail isn't gated by one long reciprocal
                        H = NCHUNK // 2
                        for h in range(2):
                            hs = slice(h * H, (h + 1) * H)
                            nc.vector.reciprocal(rdb[:, hs], dsb[:, hs])
                            for ot in range(CT):
                                nsl_h = slice(nb * NCHUNK + h * H,
                                              nb * NCHUNK + (h + 1) * H)
                                y = wp.tile([128, NCHUNK], F32, tag="y", bufs=6, name="y")
                                nc.vector.tensor_mul(y[:, hs], zsb[ot][:, hs], rdb[:, hs])
                                nc.vector.scalar_tensor_tensor(
                                    y[:, hs], in0=y[:, hs], scalar=bp[:, ot:ot + 1],
                                    in1=xt[ot][:, nsl_h], op0=ALU.add, op1=ALU.add)
                                nc.sync.dma_start(out_d[ot * 128:(ot + 1) * 128, nsl_h],
                                                  y[:, hs])
    nc.compile()
    return nc


_NC = None


def _get_nc():
    global _NC
    if _NC is None:
        _NC = _build()
    return _NC


def prepare_shared(gn_w, gn_b, qkv_w, qkv_b, proj_w, proj_b):
    wqkvT = np.ascontiguousarray(np.asarray(qkv_w, np.float32).T)      # [C, 3C]
    wpT = np.ascontiguousarray(np.asarray(proj_w, np.float32).T)       # [C, C]
    # fp8 proj weights laid out [128, c-tile, C_out]
    wpT8 = np.ascontiguousarray(
        wpT.reshape(CT, 128, C).transpose(1, 0, 2).reshape(128, 2 * C)
    ).astype(ml_dtypes.float8_e4m3)
    qkv_b = np.asarray(qkv_b, np.float32)
    bqk = np.ascontiguousarray(qkv_b[:2 * C].reshape(4, 128).T)        # [128, 4]
    bvrow = np.ascontiguousarray(qkv_b[2 * C:].reshape(1, C))          # [1, C]
    bp = np.ascontiguousarray(np.asarray(proj_b, np.float32).reshape(CT, 128).T)
    gamma = np.ascontiguousarray(np.asarray(gn_w, np.float32).reshape(CT, 128).T)
    beta = np.ascontiguousarray(np.asarray(gn_b, np.float32).reshape(CT, 128).T)

    # group selectors: channel c -> group c // GSIZE
    sel = np.zeros((128, 2 * GROUPS), np.float32)
    selb = np.zeros((GROUPS, C), np.float32)
    for t in range(CT):
        for p in range(128):
            g = (t * 128 + p) // GSIZE
            sel[p, t * GROUPS + g] = 1.0
            selb[g, t * 128 + p] = 1.0

    return {
        "wqkvT": wqkvT, "wpT8": wpT8, "bqk": bqk, "bvrow": bvrow, "bp": bp,
        "gamma": gamma, "beta": beta, "sel": sel, "selb": selb,
        "ones": np.ones((128, 128), np.float32),
    }


def kernel(x, gn_w, gn_b, qkv_w, qkv_b, proj_w, proj_b):
    x = np.asarray(x, dtype=np.float32)
    b = x.shape[0]
    assert b == 8 and x.shape[1] == C
    xs = x.reshape(b, C, N)

    nc = _get_nc()
    shared = prepare_shared(gn_w, gn_b, qkv_w, qkv_b, proj_w, proj_b)
    in_maps = [dict(shared, x=np.ascontiguousarray(xs[i])) for i in range(b)]
    res = run_bass_kernel_spmd(nc, in_maps, core_ids=list(range(8)))
    out = np.stack([res.results[i]["out"] for i in range(b)])
    return out.reshape(x.shape).astype(np.float32)


# revision 24
# speedup vs baseline: 1.2158x; 1.2158x over previous
"""AttentionBlock kernel for Trainium2, data-parallel over batch on 8 NeuronCores.

Per-core computation (one batch element, x_b: [256, 4096] = [C, H*W]):
  GroupNorm(8 groups) folded into the QKV projection:
    xn = x*scale_c + shift_c   (per-channel affine from group stats)
    qkv = W_qkv xn + b  ==  (W_qkv * scale_c) x + (W_qkv shift + b)
  Everything heavy runs as fp8e4m3 DoubleRow matmuls: each instruction
  contracts 2 k-tiles (K=256) at ~1.7x the f32r rate.  During the x DMA,
  the fp8 cast of x doubles as the GN sum (ACT accum_out) while the DVE
  computes the sum of squares (tensor_tensor_reduce), so GN stats are
  ready as soon as x lands.  q,k land in fp8 [128, 2, N]; v goes out
  transposed as vT8 [128, MT, C] with the v-bias added via a precomputed
  broadcast row and scaled by 1/8 so the attention numerator stays in
  e4m3 range.  QKV matmul/eviction units are interleaved so attention can
  start while late eviction chunks still drain.
  Attention core (per 512-token query chunk nb, per key-tile pair j):
    S'[m,n] = sum_c k8[c,m] q8[c,n]   (1 DoubleRow matmul per m-tile)
    P' = exp(S'/16 - 2.5) -> fp8      (the -2.5 bias keeps P' in e4m3
                                       range and cancels in P'/d)
    out[c,n] += vT8 pair @ P' pair    (DoubleRow over m-tile pairs)
    d[n]    += ones8 pair @ P' pair   (ones8 = 0.125 on all 128 rows: the
                                       denominator lands pre-broadcast
                                       across partitions and pre-scaled)
  S/exp emission runs 2 pairs ahead of out/d so the ACT latency hides, and
  the prefetch continues across nb boundaries (the epilogue of chunk nb is
  emitted between the first S pairs of chunk nb+1).
  Normalization is deferred past the proj matmul (it commutes):
    y = (proj8(att8) * (1/d)) + proj_b + x
"""

import sys

sys.path.insert(0, "/opt/trn_rl_repo")

import ml_dtypes
import numpy as np

import concourse.bass as bass  # noqa: F401
import concourse.mybir as mybir
import concourse.tile as tile
from concourse import bacc
from concourse.bass_utils import run_bass_kernel_spmd

F32 = mybir.dt.float32
F32R = mybir.dt.float32r
F8 = mybir.dt.float8e4
DR = mybir.MatmulPerfMode.DoubleRow
AF = mybir.ActivationFunctionType
ALU = mybir.AluOpType

C = 256
N = 4096
GROUPS = 8
EPS = 1e-5
CT = 2          # channel tiles of 128
MT = 32         # m (key/token) tiles of 128
NB = 8          # n (query/token) chunks of 512
NCHUNK = 512
SCALE = 1.0 / 16.0  # 1/sqrt(C)
EXPB = -2.5         # exp bias: P' = exp(S/16 - 2.5), keeps fp8e4 in range
VSCALE = 0.125      # v scaled by 1/8 into fp8 so att=P'@v stays under 240;
                    # ones8 = VSCALE so the same factor lands in d and cancels
GSIZE = C // GROUPS
GN_COUNT = float(GSIZE * N)
XCH = 2         # x DMA/stat chunks per c-tile
XCW = N // XCH  # 2048
NPAIR = MT // 2
PREF = 2        # S/exp pairs emitted ahead of out/d accumulation


def _build():
    nc = bacc.Bacc("TRN2", target_bir_lowering=False)

    x_d = nc.declare_dram_parameter("x", [C, N], F32, isOutput=False)
    wqkvT_d = nc.declare_dram_parameter("wqkvT", [C, 3 * C], F32R, isOutput=False)
    wpT8_d = nc.declare_dram_parameter("wpT8", [128, 2 * C], F8, isOutput=False)
    bqk_d = nc.declare_dram_parameter("bqk", [128, 4], F32, isOutput=False)
    bvrow_d = nc.declare_dram_parameter("bvrow", [1, C], F32, isOutput=False)
    bp_d = nc.declare_dram_parameter("bp", [128, 2], F32, isOutput=False)
    gamma_d = nc.declare_dram_parameter("gamma", [128, 2], F32, isOutput=False)
    beta_d = nc.declare_dram_parameter("beta", [128, 2], F32, isOutput=False)
    sel_d = nc.declare_dram_parameter("sel", [128, 2 * GROUPS], F32, isOutput=False)
    selb_d = nc.declare_dram_parameter("selb", [GROUPS, C], F32, isOutput=False)
    ones_d = nc.declare_dram_parameter("ones", [128, 128], F32R, isOutput=False)
    out_d = nc.declare_dram_parameter("out", [C, N], F32, isOutput=True)
    dbg_d = nc.declare_dram_parameter("dbg", [1, 2], F32, isOutput=True)

    with tile.TileContext(nc) as tc:
        with (
            tc.tile_pool(name="const", bufs=1) as cp,
            tc.tile_pool(name="work", bufs=1) as wp,
            nc.allow_low_precision("f32r accumulators hold exact f32 bits"),
        ):
            ones = cp.tile([128, 128], F32R, name="ones", tag="ones")
            nc.sync.dma_start(ones[:], ones_d[:])
            ones8 = cp.tile([128, 2, 128], F8, name="ones8", tag="ones8")
            nc.any.memset(ones8[:], VSCALE)
            bias_exp = cp.tile([128, 1], F32, name="bias_exp", tag="bias_exp")
            nc.any.memset(bias_exp[:], EXPB)
            # dummy activations preload the Sqrt/Exp ACT tables while the x
            # DMA is in flight; DMA'd to a debug output so they survive DCE
            dumm = cp.tile([1, 1], F32, name="dumm", tag="dumm")
            nc.any.memset(dumm[:], 1.0)
            dumo = cp.tile([1, 2], F32, name="dumo", tag="dumo")
            nc.scalar.activation(dumo[:, 0:1], dumm[:], AF.Sqrt)
            nc.scalar.activation(dumo[:, 1:2], dumm[:], AF.Exp)
            nc.sync.dma_start(dbg_d[:], dumo[:])
            # ---- x loads, chunk-major so casts/stats start early ----
            xt = [cp.tile([128, N], F32, name=f"x{t}", tag=f"x{t}") for t in range(CT)]
            for ch in range(XCH):
                for t in range(CT):
                    nc.sync.dma_start(xt[t][:, ch * XCW:(ch + 1) * XCW],
                                      x_d[t * 128:(t + 1) * 128, ch * XCW:(ch + 1) * XCW])
            # ---- fp8 cast doubles as GN sum (ACT accum); DVE does sum-sq ----
            x8 = cp.tile([128, CT, N], F8, name="x8", tag="x8")
            stats = [cp.tile([128, 2 * XCH], F32, name=f"stats{t}", tag=f"stats{t}")
                     for t in range(CT)]
            for ch in range(XCH):
                for t in range(CT):
                    xv = xt[t][:, ch * XCW:(ch + 1) * XCW]
                    nc.vector.tensor_reduce(stats[t][:, ch:ch + 1], xv,
                                            mybir.AxisListType.X, ALU.add)
                    scratch = wp.tile([128, XCW], F32, tag="scratch", name="scratch")
                    nc.scalar.activation(scratch[:], xv, AF.Square,
                                         accum_out=stats[t][:, XCH + ch:XCH + ch + 1])

            # ---- remaining loads ----
            wT = []
            for t in range(CT):
                wtile = cp.tile([128, 3 * C], F32R, name=f"wT{t}", tag=f"wT{t}")
                nc.sync.dma_start(wtile[:], wqkvT_d[t * 128:(t + 1) * 128, :])
                wT.append(wtile)
            wp8t = cp.tile([128, 2, C], F8, name="wpT8", tag="wpT8")
            nc.sync.dma_start(wp8t[:], wpT8_d[:])
            bqk = cp.tile([128, 4], F32, name="bqk", tag="bqk")
            nc.sync.dma_start(bqk[:], bqk_d[:])
            bvrow = cp.tile([1, C], F32, name="bvrow", tag="bvrow")
            nc.sync.dma_start(bvrow[:], bvrow_d[:])
            bp = cp.tile([128, 2], F32, name="bp", tag="bp")
            nc.sync.dma_start(bp[:], bp_d[:])
            gamma = cp.tile([128, 2], F32, name="gamma", tag="gamma")
            nc.sync.dma_start(gamma[:], gamma_d[:])
            beta = cp.tile([128, 2], F32, name="beta", tag="beta")
            nc.sync.dma_start(beta[:], beta_d[:])
            sel = cp.tile([128, 2 * GROUPS], F32, name="sel", tag="sel")
            nc.sync.dma_start(sel[:], sel_d[:])
            selb = cp.tile([GROUPS, C], F32, name="selb", tag="selb")
            nc.sync.dma_start(selb[:], selb_d[:])

            # ---- setup-phase PSUM pool (closed before the QKV pool) ----
            with tc.tile_pool(name="ps0", bufs=2, space="PSUM") as ps0:
                # dummy matmuls keep the PE HAM-warm while DMA + GN stats run
                for wi in range(56):
                    wps = ps0.tile([128, 128], F32, tag="warm", name="wps")
                    nc.tensor.matmul(wps[:], ones[:], ones[:], start=True, stop=True)
                g_ps = ps0.tile([GROUPS, 2 * XCH], F32, tag="small", name="g_ps")
                nc.tensor.matmul(g_ps[:], sel[:, 0:GROUPS], stats[0][:], start=True, stop=False)
                nc.tensor.matmul(g_ps[:], sel[:, GROUPS:2 * GROUPS], stats[1][:], start=False, stop=True)
                # per-group mean / rstd on partitions 0..7
                g_mr = cp.tile([GROUPS, 2], F32, name="g_mr", tag="g_mr")
                gtmp = cp.tile([GROUPS, 5], F32, name="gtmp", tag="gtmp")
                g_sb = cp.tile([GROUPS, 2 * XCH], F32, name="g_sb", tag="g_sb")
                nc.vector.tensor_copy(g_sb[:], g_ps[:])
                nc.vector.tensor_add(gtmp[:, 3:4], g_sb[:, 0:1], g_sb[:, 1:2])
                nc.vector.tensor_add(gtmp[:, 4:5], g_sb[:, 2:3], g_sb[:, 3:4])
                nc.vector.tensor_scalar_mul(g_mr[:, 0:1], gtmp[:, 3:4], 1.0 / GN_COUNT)
                nc.vector.tensor_scalar_mul(gtmp[:, 0:1], gtmp[:, 4:5], 1.0 / GN_COUNT)
                nc.vector.tensor_mul(gtmp[:, 1:2], g_mr[:, 0:1], g_mr[:, 0:1])
                nc.vector.tensor_sub(gtmp[:, 2:3], gtmp[:, 0:1], gtmp[:, 1:2])
                gvar = cp.tile([GROUPS, 1], F32, name="gvar", tag="gvar")
                nc.vector.tensor_scalar_add(gvar[:], gtmp[:, 2:3], EPS)
                gstd = cp.tile([GROUPS, 1], F32, name="gstd", tag="gstd")
                nc.scalar.activation(gstd[:], gvar[:], AF.Sqrt)
                nc.vector.reciprocal(g_mr[:, 1:2], gstd[:])

                # broadcast group mean/rstd to per-channel scale/shift
                scale_t = []
                shift_t = []
                for t in range(CT):
                    mr_ps = ps0.tile([128, 2], F32, tag="small", name="mr_ps")
                    nc.tensor.matmul(mr_ps[:], selb[:, t * 128:(t + 1) * 128], g_mr[:],
                                     start=True, stop=True)
                    mr = cp.tile([128, 2], F32, name=f"mr{t}", tag=f"mr{t}")
                    nc.vector.tensor_copy(mr[:], mr_ps[:])
                    sc = cp.tile([128, 1], F32, name=f"scale{t}", tag=f"scale{t}")
                    nc.vector.tensor_mul(sc[:], mr[:, 1:2], gamma[:, t:t + 1])
                    tmp = cp.tile([128, 1], F32, name=f"mscale{t}", tag=f"mscale{t}")
                    nc.vector.tensor_mul(tmp[:], mr[:, 0:1], sc[:])
                    # shift duplicated to 2 cols: f32r matmuls need even N
                    sh = cp.tile([128, 2], F32R, name=f"shift{t}", tag=f"shift{t}")
                    nc.vector.tensor_sub(sh[:, 0:1], beta[:, t:t + 1], tmp[:])
                    nc.vector.tensor_sub(sh[:, 1:2], beta[:, t:t + 1], tmp[:])
                    scale_t.append(sc)
                    shift_t.append(sh)

                # adjusted fp8 qkv weights: wadj8[c, t, o] = wT[c, o] * scale_c
                wadj8 = cp.tile([128, 2, 3 * C], F8, name="wadj8", tag="wadj8")
                for t in range(CT):
                    nc.vector.tensor_scalar_mul(wadj8[:, t:t + 1, :],
                                                wT[t][:].bitcast(F32), scale_t[t][:])
                # x8 casts emitted only now: in-order queues would otherwise
                # run them before the GN-chain ops they don't depend on
                for ch in range(XCH):
                    for t in range(CT):
                        xv = xt[t][:, ch * XCW:(ch + 1) * XCW]
                        cdst = x8[:, t:t + 1, ch * XCW:(ch + 1) * XCW]
                        if ch == 0:
                            nc.scalar.copy(cdst, xv)
                        else:
                            nc.vector.tensor_copy(cdst, xv)
                # q/k bias: btot[o] = qkv_b[o] + sum_c wT[c,o]*shift_c  (o in 0..512)
                bias_ps = ps0.tile([128, 4, 2], F32, tag="small", name="bias_ps")
                for ot in range(4):
                    for t in range(CT):
                        nc.tensor.matmul(bias_ps[:, ot:ot + 1, :],
                                         wT[t][:, ot * 128:(ot + 1) * 128],
                                         shift_t[t][:],
                                         start=(t == 0), stop=(t == CT - 1))
                btot = cp.tile([128, 4], F32, name="btot", tag="btot")
                nc.vector.tensor_add(btot[:], bias_ps[:, :, 0:1], bqk[:])
                # v bias row doubled: bvtot2[1, 2C] then broadcast [128, 2C] * VSCALE
                bv_ps = ps0.tile([2, C], F32, tag="small", name="bv_ps")
                for t in range(CT):
                    nc.tensor.matmul(bv_ps[:], shift_t[t][:], wT[t][:, 2 * C:3 * C],
                                     start=(t == 0), stop=(t == CT - 1))
                bvtot2 = cp.tile([1, 2 * C], F32R, name="bvtot2", tag="bvtot2")
                for h in range(2):
                    nc.vector.tensor_add(bvtot2[:, h * C:(h + 1) * C], bv_ps[0:1, :], bvrow[:])
                bv2_ps = ps0.tile([128, 2 * C], F32, tag="bv2", name="bv2_ps")
                nc.tensor.matmul(bv2_ps[:], ones[0:1, :], bvtot2[:], start=True, stop=True)
                bv2s = cp.tile([128, 2 * C], F32, name="bv2s", tag="bv2s")
                nc.vector.tensor_scalar_mul(bv2s[:], bv2_ps[:], VSCALE)

            with tc.tile_pool(name="ps1", bufs=1, space="PSUM") as ps1:
                # ---- QKV projections (fp8 DR): q,k -> fp8 [128, 2, N] ----
                q8 = cp.tile([128, CT, N], F8, name="q8", tag="q8")
                k8 = cp.tile([128, CT, N], F8, name="k8", tag="k8")
                vT8 = cp.tile([128, MT, C], F8, name="vT8", tag="vT8")
                dests = [(q8, 0), (q8, 1), (k8, 0), (k8, 1)]

                def emit_qk(ot, mcp, eng):
                    qk_ps = ps1.tile([128, 2 * NCHUNK], F32, tag="qk", bufs=3, name="qk_ps")
                    for half in range(2):
                        mc = 2 * mcp + half
                        nc.tensor.matmul(qk_ps[:, half * NCHUNK:(half + 1) * NCHUNK],
                                         wadj8[:, :, ot * 128:(ot + 1) * 128],
                                         x8[:, :, mc * NCHUNK:(mc + 1) * NCHUNK],
                                         start=True, stop=True, perf_mode=DR)
                    dtile, dt_ = dests[ot]
                    dst = dtile[:, dt_:dt_ + 1, 2 * mcp * NCHUNK:(2 * mcp + 2) * NCHUNK]
                    if eng == "act":
                        nc.scalar.activation(dst, qk_ps[:], AF.Identity,
                                             bias=btot[:, ot:ot + 1])
                    else:
                        nc.vector.tensor_scalar_add(dst, qk_ps[:], btot[:, ot:ot + 1])

                def emit_vt(mtp):
                    vt_ps = ps1.tile([128, 2 * C], F32, tag="vt", bufs=2, name="vt_ps")
                    for half in range(2):
                        mt = 2 * mtp + half
                        nc.tensor.matmul(vt_ps[:, half * C:(half + 1) * C],
                                         x8[:, :, mt * 128:(mt + 1) * 128],
                                         wadj8[:, :, 2 * C:3 * C],
                                         start=True, stop=True, perf_mode=DR)
                    nc.vector.scalar_tensor_tensor(
                        vT8[:, 2 * mtp:2 * mtp + 2, :], in0=vt_ps[:], scalar=VSCALE,
                        in1=bv2s[:], op0=ALU.mult, op1=ALU.add)

                # qk units first (q chunk0 and k earliest, on the faster DVE
                # evictor), then vT units; attention consumes them in order
                emit_qk(0, 0, "dve"); emit_qk(1, 0, "dve")
                emit_qk(2, 0, "dve"); emit_qk(3, 0, "dve")
                emit_qk(2, 1, "act"); emit_qk(3, 1, "act")
                emit_qk(2, 2, "act"); emit_qk(3, 2, "act")
                emit_qk(2, 3, "act"); emit_qk(3, 3, "act")
                emit_qk(0, 1, "act"); emit_qk(1, 1, "act")
                emit_qk(0, 2, "act"); emit_qk(1, 2, "act")
                emit_qk(0, 3, "dve"); emit_qk(1, 3, "dve")
                for mtp in range(MT // 2):
                    emit_vt(mtp)

            with tc.tile_pool(name="ps", bufs=1, space="PSUM") as ps:
                # ---- attention: fp8 DoubleRow core, software-pipelined ----
                total = NB * NPAIR
                p8_of = {}

                def emit_s_exp(idx):
                    nb, j = divmod(idx, NPAIR)
                    nsl = slice(nb * NCHUNK, (nb + 1) * NCHUNK)
                    p8 = wp.tile([128, 2, NCHUNK], F8, tag="p", bufs=4, name="p8")
                    s_ps = ps.tile([128, 2 * NCHUNK], F32, tag="s", bufs=2, name="s_ps")
                    for i in range(2):
                        mb = 2 * j + i
                        nc.tensor.matmul(s_ps[:, i * NCHUNK:(i + 1) * NCHUNK],
                                         k8[:, :, mb * 128:(mb + 1) * 128],
                                         q8[:, :, nsl],
                                         start=True, stop=True, perf_mode=DR)
                    # one ACT exp per m-tile pair: halves ACT instruction count
                    nc.scalar.activation(p8[:], s_ps[:], AF.Exp,
                                         bias=bias_exp[:, 0:1], scale=SCALE)
                    p8_of[idx] = p8

                emitted = 0

                def prefetch(upto):
                    nonlocal emitted
                    while emitted <= min(upto, total - 1):
                        emit_s_exp(emitted)
                        emitted += 1

                for nb in range(NB):
                    nsl = slice(nb * NCHUNK, (nb + 1) * NCHUNK)
                    out_ps = [ps.tile([128, NCHUNK], F32, tag="out", bufs=3, name=f"outp{_t}")
                              for _t in range(CT)]
                    d_ps = ps.tile([128, NCHUNK], F32, tag="d", bufs=1, name="d_ps")
                    base = nb * NPAIR
                    for j in range(NPAIR):
                        idx = base + j
                        prefetch(idx + PREF)
                        p_cur = p8_of.pop(idx)
                        first, last = (j == 0), (j == NPAIR - 1)
                        for t in range(CT):
                            nc.tensor.matmul(out_ps[t][:],
                                             vT8[:, 2 * j:2 * j + 2, t * 128:(t + 1) * 128],
                                             p_cur[:], start=first, stop=last,
                                             perf_mode=DR)
                        nc.tensor.matmul(d_ps[:], ones8[:], p_cur[:],
                                         start=first, stop=last, perf_mode=DR)
                    # ---- epilogue: evict, normalize via deferred 1/d, proj ----
                    last_nb = (nb == NB - 1)
                    att8 = wp.tile([128, CT, NCHUNK], F8, tag="att", bufs=2, name="att8")
                    for t in range(CT):
                        nc.vector.tensor_copy(att8[:, t:t + 1, :], out_ps[t][:])
                    dsb = wp.tile([128, NCHUNK], F32, tag="dsb", bufs=2, name="dsb")
                    if last_nb:  # ACT is idle at the tail; shortens the exit chain
                        nc.scalar.copy(dsb[:], d_ps[:])
                    else:
                        nc.vector.tensor_copy(dsb[:], d_ps[:])
                    zsb = []
                    for ot in range(CT):
                        z_ps = ps.tile([128, NCHUNK], F32, tag="out", bufs=3, name="z_ps")
                        nc.tensor.matmul(z_ps[:],
                                         wp8t[:, :, ot * 128:(ot + 1) * 128],
                                         att8[:], start=True, stop=True, perf_mode=DR)
                        zt = wp.tile([128, NCHUNK], F32, tag="z", bufs=3, name="zsb")
                        if last_nb:
                            nc.scalar.copy(zt[:], z_ps[:])
                        else:
                            nc.vector.tensor_copy(zt[:], z_ps[:])
                        zsb.append(zt)
                    rdb = wp.tile([128, NCHUNK], F32, tag="rdb", bufs=2, name="rdb")
                    if nb < NB - 1:
                        nc.vector.reciprocal(rdb[:], dsb[:])
                        for ot in range(CT):
                            y = wp.tile([128, NCHUNK], F32, tag="y", bufs=6, name="y")
                            nc.vector.tensor_mul(y[:], zsb[ot][:], rdb[:])
                            nc.vector.scalar_tensor_tensor(
                                y[:], in0=y[:], scalar=bp[:, ot:ot + 1],
                                in1=xt[ot][:, nsl], op0=ALU.add, op1=ALU.add)
                            nc.sync.dma_start(out_d[ot * 128:(ot + 1) * 128, nsl], y[:])
                    else:
                        # last chunk: halve the normalize/residual chain so the
                        # kernel tail isn't gated by one long reciprocal
                        H = NCHUNK // 2
                        for h in range(2):
                            hs = slice(h * H, (h + 1) * H)
                            nc.vector.reciprocal(rdb[:, hs], dsb[:, hs])
                            for ot in range(CT):
                                nsl_h = slice(nb * NCHUNK + h * H,
                                              nb * NCHUNK + (h + 1) * H)
                                y = wp.tile([128, NCHUNK], F32, tag="y", bufs=6, name="y")
                                nc.vector.tensor_mul(y[:, hs], zsb[ot][:, hs], rdb[:, hs])
                                nc.vector.scalar_tensor_tensor(
                                    y[:, hs], in0=y[:, hs], scalar=bp[:, ot:ot + 1],
                                    in1=xt[ot][:, nsl_h], op0=ALU.add, op1=ALU.add)
                                nc.sync.dma_start(out_d[ot * 128:(ot + 1) * 128, nsl_h],
                                                  y[:, hs])
    nc.compile()
    return nc


_NC = None


def _get_nc():
    global _NC
    if _NC is None:
        _NC = _build()
    return _NC


def prepare_shared(gn_w, gn_b, qkv_w, qkv_b, proj_w, proj_b):
    wqkvT = np.ascontiguousarray(np.asarray(qkv_w, np.float32).T)      # [C, 3C]
    wpT = np.ascontiguousarray(np.asarray(proj_w, np.float32).T)       # [C, C]
    # fp8 proj weights laid out [128, c-tile, C_out]
    wpT8 = np.ascontiguousarray(
        wpT.reshape(CT, 128, C).transpose(1, 0, 2).reshape(128, 2 * C)
    ).astype(ml_dtypes.float8_e4m3)
    qkv_b = np.asarray(qkv_b, np.float32)
    bqk = np.ascontiguousarray(qkv_b[:2 * C].reshape(4, 128).T)        # [128, 4]
    bvrow = np.ascontiguousarray(qkv_b[2 * C:].reshape(1, C))          # [1, C]
    bp = np.ascontiguousarray(np.asarray(proj_b, np.float32).reshape(CT, 128).T)
    gamma = np.ascontiguousarray(np.asarray(gn_w, np.float32).reshape(CT, 128).T)
    beta = np.ascontiguousarray(np.asarray(gn_b, np.float32).reshape(CT, 128).T)

    # group selectors: channel c -> group c // GSIZE
    sel = np.zeros((128, 2 * GROUPS), np.float32)
    selb = np.zeros((GROUPS, C), np.float32)
    for t in range(CT):
        for p in range(128):
            g = (t * 128 + p) // GSIZE
            sel[p, t * GROUPS + g] = 1.0
            selb[g, t * 128 + p] = 1.0

    return {
        "wqkvT": wqkvT, "wpT8": wpT8, "bqk": bqk, "bvrow": bvrow, "bp": bp,
        "gamma": gamma, "beta": beta, "sel": sel, "selb": selb,
        "ones": np.ones((128, 128), np.float32),
    }


def kernel(x, gn_w, gn_b, qkv_w, qkv_b, proj_w, proj_b):
    x = np.asarray(x, dtype=np.float32)
    b = x.shape[0]
    assert b == 8 and x.shape[1] == C
    xs = x.reshape(b, C, N)

    nc = _get_nc()
    shared = prepare_shared(gn_w, gn_b, qkv_w, qkv_b, proj_w, proj_b)
    in_maps = [dict(shared, x=np.ascontiguousarray(xs[i])) for i in range(b)]
    res = run_bass_kernel_spmd(nc, in_maps, core_ids=list(range(8)))
    out = np.stack([res.results[i]["out"] for i in range(b)])
    return out.reshape(x.shape).astype(np.float32)
